# revision 1
# baseline (speedup 1.0000x reference)
"""CARAFE upsampling kernel for 8 Trainium2 NeuronCores.

Problem (hardcoded): features (2,256,128,128) f32, masks (2,25,256,256) f32,
out (2,256,256,256) f32.  K=5, G=1, scale=2 (CARAFE content-aware upsample).

Strategy
--------
Sharding: 8 cores = batch(2) x H-half(2) x W-half(2).  Each core owns the
full C=256 and a 64x64 source patch (128x128 output patch) with a 2-pixel
feature halo (sliced with halo / zero-padded on host).

Compute: the 25-tap dynamic-filter sum becomes PSUM-accumulated TensorE
matmuls.  For source-row pair si, channel half ch, tap row dy, and output
column half h:

    out[c, (a, jj)] += featT[x', y=si+dy, c]^T  @  band[x', (a, jj)]

The band matrix (built host-side) holds mask values along x = jj//2 + dx
diagonals, zeros elsewhere.  Splitting the j-range in half (h) keeps the
contraction at K=36 (32 + 4 halo) instead of 68, nearly halving the band
bytes shipped from HBM.  The h=1 window (x in [32,68)) is x-REVERSED on the
host so both halves contract at partition base 0 (the contraction sum is
order-invariant); partition-offset matmuls crash the runtime here.  Each
matmul writes a contiguous 128-col PSUM slice (h-major); a single
start=True per PSUM bank clears has_written for the whole bank, so h=1's
first matmul (start=False) lands on cleared bits and overwrites.  M=128
channels, N=128 pixels/matmul; fp16 operands (rel err ~3.5e-4), fp32 PSUM.

Output leaves in native (c, i, j) layout via a PSUM->SBUF copy that
unscrambles (h, a, jj) -> (a, j), staged 8 si at a time into 1 MiB stores.
DMAs are batched per 8-si group to amortize HWDGE/sequencer fixed costs;
features become fully SBUF-resident by group 2.

TimelineSim cost model: ~82 us/core; PE busy ~70 us, DMA ~70 us.
"""

import numpy as np

import concourse.bacc as bacc
import concourse.bass as bass
import concourse.mybir as mybir
import concourse.tile as tile
from concourse.bass_utils import run_bass_kernel_spmd

FP16 = mybir.dt.float16
F32 = mybir.dt.float32

N_CORES = 8
C = 256
SI = 64
SX = 64
YR = SI + 4
XW = 36
OI = 2 * SI
OJ = 2 * SX
GROUP = 8
NG = SI // GROUP

_CACHED_NC = None
TRACE = False
_LAST_RESULTS = None


def _build_nc():
    nc = bacc.Bacc(None, target_bir_lowering=False, debug=False)

    # [hblock, x', y, c]: block 0 = x 0:36; block 1 = x 67..32 (reversed)
    featT_d = nc.dram_tensor("featT", [2, XW, YR, C], FP16, kind="ExternalInput")
    # [h, x', si, dy, (a jj)]; h=1 x'-axis reversed to match featT block 1
    bands_d = nc.dram_tensor("bands", [2, XW, SI, 5, 128], FP16, kind="ExternalInput")
    out_d = nc.dram_tensor("out", [C, OI, OJ], F32, kind="ExternalOutput")

    with tile.TileContext(nc) as tc:
        with (
            tc.tile_pool(name="feat", bufs=1) as fpool,
            tc.tile_pool(name="bands", bufs=3) as bpool,
            tc.tile_pool(name="psum", bufs=8, space=bass.MemorySpace.PSUM) as ppool,
            tc.tile_pool(name="stage", bufs=4) as spool,
        ):
            ftiles = [
                fpool.tile([XW, YR * C], FP16, tag=f"ft{h}", name=f"ft{h}")
                for h in range(2)
            ]
            fdone = [0, 0]

            def load_feat_rows(upto, h):
                nonlocal fdone
                if upto <= fdone[h]:
                    return
                nc.sync.dma_start(
                    ftiles[h][:, fdone[h] * C : upto * C],
                    featT_d[h, :, fdone[h] : upto].rearrange("x y c -> x (y c)"),
                )
                fdone[h] = upto

            for g in range(NG):
                btiles = [
                    bpool.tile([XW, GROUP * 640], FP16, tag=f"bt{h}", name=f"bt{h}")
                    for h in range(2)
                ]
                for h in range(2):
                    # group 0: halve the first transfers so matmuls start early
                    # (per-DMA fixed costs dominate here, so only one split)
                    splits = [(0, 4), (4, 8)] if g == 0 else [(0, GROUP)]
                    for s0, s1 in splits:
                        src = bands_d[h, :, g * GROUP + s0 : g * GROUP + s1]
                        nc.sync.dma_start(
                            btiles[h][:, s0 * 640 : s1 * 640],
                            src.rearrange("x s d j -> x (s d j)"),
                        )
                        if g == 0:
                            load_feat_rows(s1 + 4, h)
                    if g == 0:
                        load_feat_rows(GROUP * 2 + 4, h)
                    else:
                        load_feat_rows(YR, h)
                for ch in range(2):
                    stg = spool.tile([128, GROUP * 256], F32)
                    for sl in range(GROUP):
                        si = g * GROUP + sl
                        # psum columns are h-major (contiguous per matmul);
                        # the copy below unscrambles to (a, j) order
                        ps = ppool.tile([128, 256], F32)
                        for dy in range(5):
                            yoff = (si + dy) * C + ch * 128
                            for h in range(2):
                                boff = sl * 640 + dy * 128
                                # start once per bank (clears has_written for
                                # the WHOLE bank); h=1's first write lands on
                                # cleared bits and overwrites
                                nc.tensor.matmul(
                                    ps[:, 128 * h : 128 * h + 128],
                                    ftiles[h][:, yoff : yoff + 128],
                                    btiles[h][:, boff : boff + 128],
                                    start=(dy == 0 and h == 0),
                                    stop=(dy == 4 and h == 1),
                                    skip_group_check=True,
                                )
                        # stg col = a*128 + h*64 + jj, iterated in the psum's
                        # (h, a, jj) source order
                        dst = stg[:, sl * 256 : (sl + 1) * 256].rearrange(
                            "p (a h j) -> p h a j", a=2, h=2
                        )
                        if ch == 0:
                            nc.vector.tensor_copy(dst, ps[:])
                        else:
                            nc.scalar.copy(dst, ps[:])
                    odst = out_d[
                        ch * 128 : (ch + 1) * 128,
                        g * 2 * GROUP : (g + 1) * 2 * GROUP,
                        :,
                    ]
                    nc.scalar.dma_start(odst.rearrange("c a j -> c (a j)"), stg[:])

    nc.compile()
    return nc


def _get_nc():
    global _CACHED_NC
    if _CACHED_NC is None:
        _CACHED_NC = _build_nc()
    return _CACHED_NC


def _prep_core_inputs(features: np.ndarray, masks: np.ndarray):
    fp = np.pad(features, ((0, 0), (0, 0), (2, 2), (2, 2)))

    sjl = np.arange(32)
    in_maps = []
    for core in range(N_CORES):
        n, hb, wb = core // 4, (core // 2) % 2, core % 2

        fsl = fp[n, :, hb * SI : hb * SI + YR, wb * SX : wb * SX + SX + 4]
        featT = np.ascontiguousarray(fsl.transpose(2, 1, 0))  # (x, y, c)
        fA = featT[0:36]
        fB = featT[32:68][::-1]  # x reversed
        featT2 = np.stack([fA, fB]).astype(np.float16)

        msl = masks[n, :, hb * OI : (hb + 1) * OI, wb * OJ : (wb + 1) * OJ]
        m6 = msl.reshape(5, 5, SI, 2, SX, 2)
        bh = np.zeros((2, XW, SI, 5, 2, 32, 2), dtype=np.float32)
        for h in range(2):
            for dx in range(5):
                mh = m6[:, dx].transpose(3, 1, 0, 2, 4)[32 * h : 32 * h + 32]
                bh[h, sjl + dx, :, :, :, sjl, :] = mh
        bh[1] = bh[1][::-1]  # mirror x' to match featT block 1
        bands = bh.astype(np.float16).reshape(2, XW, SI, 5, 128)

        in_maps.append({"featT": featT2, "bands": bands})
    return in_maps


def kernel(features: np.ndarray, masks: np.ndarray) -> np.ndarray:
    global _LAST_RESULTS
    features = np.asarray(features, dtype=np.float32)
    masks = np.asarray(masks, dtype=np.float32)

    nc = _get_nc()
    in_maps = _prep_core_inputs(features, masks)
    res = run_bass_kernel_spmd(nc, in_maps, list(range(N_CORES)), trace=TRACE)
    _LAST_RESULTS = res

    out = np.empty((2, C, 256, 256), dtype=np.float32)
    for core in range(N_CORES):
        n, hb, wb = core // 4, (core // 2) % 2, core % 2
        out[n, :, hb * OI : (hb + 1) * OI, wb * OJ : (wb + 1) * OJ] = res.results[
            core
        ]["out"]
    return out



# revision 6
# speedup vs baseline: 1.3275x; 1.3275x over previous
"""CARAFE upsampling kernel for 8 Trainium2 NeuronCores.

Problem (hardcoded): features (2,256,128,128) f32, masks (2,25,256,256) f32,
out (2,256,256,256) f32.  K=5, G=1, scale=2 (CARAFE content-aware upsample).

Strategy
--------
Sharding: 8 cores = batch(2) x H-half(2) x W-half(2).  Each core owns the
full C=256 and a 64x64 source patch (128x128 output patch) with a 2-pixel
feature halo (zero-padded on host).

Compute: ALL 25 taps of the dynamic filter are packed into a single K=120
TensorE contraction per output tile.  Tiles are (si-block B=8 source rows) x
(x-window jwin=6 source cols): the contraction axis enumerates (yrel, u) =
12 y-rows x 10 x-cols of the feature patch covering the whole tile + halo.
For tile (sb, w) and channel half ch:

    out[c, (si a jj b)] = featT[(yrel u), c]^T  @  band[(yrel u), (si jj a b)]

featT is the feature patch in [(yrel u), c] layout (host-replicated: a row
appears in ~1.4 si-blocks and a column in ~1.6 windows -> 5.4 MB vs 2.35 MB
raw; shipping the replicated layout is what lets one matmul cover all 25
taps).  band holds mask values at (yrel=si+dy, u=jj+dx), zeros elsewhere
(25 live taps of 120 rows per column).  Since the cost of a matmul scales
only with output columns (K is free), this cuts PE time 5x vs per-dy
accumulation: 32768 psum cols total ~ 14 us.

11 x-windows: 10 full (jwin=6, N=192 psum cols) + 1 edge (jwin=4, N=128,
x-window overlapping the previous one; band zeros mask the overlap).  Two
windows share a PSUM bank (384 cols); the second matmul uses start=False
and lands on cleared has_written bits, overwriting its column range.

Output: fp16 DRAM tensor (halves store traffic; adds ~1e-4 rel err).
PSUM->SBUF unscramble copies (psum (si jj a b) -> stage (si a j)) run on
DVE for ch0 and ACT for ch1; stages are 4 KB/partition stores with
contiguous 4 KB runs per channel -> full DMA bandwidth.

TimelineSim cost model: DMA ~49 us (feat 5.4 MB + bands 3.9 MB + out
8.4 MB fp16 at 360 GB/s) is the bottleneck; PE ~14-27 us hides under it.
"""

import numpy as np

import concourse.bacc as bacc
import concourse.bass as bass
import concourse.mybir as mybir
import concourse.tile as tile
from concourse.bass_utils import run_bass_kernel_spmd

FP16 = mybir.dt.float16
F32 = mybir.dt.float32

N_CORES = 8
C = 256
SI = 64           # source rows per core
SX = 64           # source cols per core
B = 8             # si-block size
NSB = SI // B     # 8 si-blocks
YW = B + 4        # 12 y-rows per block
JW = 6            # source cols per full window
W = JW + 4        # 10 x-cols per window
NWIN = 11         # 10 full + 1 edge window
KP = YW * W       # 120 contraction partitions
NCOL = 2048       # psum/stage cols per (sb, ch): 10*192 + 128
X0 = [0, 6, 12, 18, 24, 30, 36, 42, 48, 54, 58]  # window x origins (padded)

_CACHED_NC = None
TRACE = False
_LAST_RESULTS = None


def _build_nc():
    nc = bacc.Bacc(None, target_bir_lowering=False, debug=False)

    featrep_d = nc.dram_tensor("featrep", [NSB, KP, NWIN, C], FP16,
                               kind="ExternalInput")
    bands_d = nc.dram_tensor("bands", [NSB, KP, NCOL], FP16,
                             kind="ExternalInput")
    out_d = nc.dram_tensor("out", [C, 2 * SI, 2 * SX], FP16,
                           kind="ExternalOutput")

    with tile.TileContext(nc) as tc:
        with (
            tc.tile_pool(name="feat", bufs=4) as fpool,
            tc.tile_pool(name="bands", bufs=4) as bpool,
            tc.tile_pool(name="psum", bufs=8, space=bass.MemorySpace.PSUM) as ppool,
            tc.tile_pool(name="stage", bufs=4) as spool,
        ):
            for sb in range(NSB):
                ftile = fpool.tile([KP, NWIN * C], FP16, tag="ft", name=f"ft{sb}")
                btile = bpool.tile([KP, NCOL], FP16, tag="bt", name=f"bt{sb}")
                nc.sync.dma_start(
                    ftile[:], featrep_d[sb].rearrange("p w c -> p (w c)"))
                nc.sync.dma_start(btile[:], bands_d[sb])

                for ch in range(2):
                    stg = spool.tile([128, NCOL], FP16)
                    stgv = stg[:].rearrange("p (s a u) -> p s a u", s=B, a=2)
                    for wp in range(6):
                        ps = ppool.tile([128, 384], F32)
                        wis = [2 * wp, 2 * wp + 1] if wp < 5 else [10]
                        for k, wi in enumerate(wis):
                            jw = JW if wi < 10 else 4
                            ncols = 4 * B * jw
                            nc.tensor.matmul(
                                ps[:, 192 * k: 192 * k + ncols],
                                ftile[:, wi * C + ch * 128: wi * C + ch * 128 + 128],
                                btile[:, wi * 192: wi * 192 + ncols],
                                start=(k == 0),
                                stop=(k == len(wis) - 1),
                                skip_group_check=True,
                            )
                        # psum cols are (w, s, a, jb); stage cols (s, a, j).
                        # One copy per subpixel-row a keeps both APs 3D.
                        for a in range(2):
                            if wp < 5:
                                src = ps[:].rearrange(
                                    "p (w s a q) -> p s a w q",
                                    w=2, s=B, a=2, q=12)[:, :, a]
                                dst = stgv[:, :, a, 24 * wp: 24 * wp + 24].rearrange(
                                    "p s (w q) -> p s w q", w=2, q=12)
                            else:
                                src = ps[:, :128].rearrange(
                                    "p (s a q) -> p s a q",
                                    s=B, a=2, q=8)[:, :, a]
                                dst = stgv[:, :, a, 120:128]
                            if ch == 0:
                                nc.vector.tensor_copy(dst, src)
                            else:
                                nc.scalar.copy(dst, src)
                    odst = out_d[ch * 128: (ch + 1) * 128,
                                 sb * 2 * B: (sb + 1) * 2 * B, :]
                    nc.scalar.dma_start(odst.rearrange("c a j -> c (a j)"), stg[:])

    nc.compile()
    return nc


def _get_nc():
    global _CACHED_NC
    if _CACHED_NC is None:
        _CACHED_NC = _build_nc()
    return _CACHED_NC


def _prep_core_inputs(features: np.ndarray, masks: np.ndarray):
    fp = np.pad(features, ((0, 0), (0, 0), (2, 2), (2, 2)))

    x0 = np.array(X0)
    u = np.arange(W)
    x_uw = u[:, None] + x0[None, :]                       # (u, w) padded x
    sbi = np.arange(NSB)
    yrel = np.arange(YW)
    y_sy = 8 * sbi[:, None] + yrel[None, :]               # (sb, yrel) padded y

    wcols = np.arange(10)[:, None, None] * 192
    acols = np.arange(2)[None, :, None]
    bcols = np.arange(2)[None, None, :]

    in_maps = []
    for core in range(N_CORES):
        n, hb, wb = core // 4, (core // 2) % 2, core % 2

        fsl = fp[n, :, hb * SI: hb * SI + SI + 4, wb * SX: wb * SX + SX + 4]
        fyxc = np.ascontiguousarray(fsl.transpose(1, 2, 0))  # (68y, 68x, 256c)
        featrep = fyxc[y_sy[:, :, None, None], x_uw[None, None, :, :], :]
        featrep = featrep.reshape(NSB, KP, NWIN, C).astype(np.float16)

        msl = masks[n, :, hb * 2 * SI: (hb + 1) * 2 * SI,
                    wb * 2 * SX: (wb + 1) * 2 * SX]
        bands = np.zeros((NSB, YW, W, NCOL), np.float32)
        for dy in range(5):
            for dx in range(5):
                M = msl[5 * dy + dx].reshape(SI, 2, SX, 2)  # (S, a, xj, b)
                for si in range(B):
                    # full windows: xj = 6w + jj, w in [0,10)
                    # col = w*192 + si*24 + a*12 + jj*2 + b
                    for jj in range(JW):
                        vals = M[si::B][:, :, jj:60:JW, :]  # (sb, a, w, b)
                        cols = wcols + si * 24 + acols * 12 + jj * 2 + bcols
                        bands[:, si + dy, jj + dx, cols.ravel()] = (
                            vals.transpose(0, 2, 1, 3).reshape(NSB, -1))
                    # edge window w=10: xj = 60 + jjloc, u = 2 + jjloc + dx
                    # col = 1920 + si*16 + a*8 + jjloc*2 + b
                    for jjl in range(4):
                        vals = M[si::B][:, :, 60 + jjl, :]  # (sb, a, b)
                        cols = 1920 + si * 16 + acols[0] * 8 + jjl * 2 + bcols[0]
                        bands[:, si + dy, 2 + jjl + dx, cols.ravel()] = (
                            vals.reshape(NSB, -1))
        bands = bands.reshape(NSB, KP, NCOL).astype(np.float16)

        in_maps.append({"featrep": featrep, "bands": bands})
    return in_maps


def kernel(features: np.ndarray, masks: np.ndarray) -> np.ndarray:
    global _LAST_RESULTS
    features = np.asarray(features, dtype=np.float32)
    masks = np.asarray(masks, dtype=np.float32)

    nc = _get_nc()
    in_maps = _prep_core_inputs(features, masks)
    res = run_bass_kernel_spmd(nc, in_maps, list(range(N_CORES)), trace=TRACE)
    _LAST_RESULTS = res

    out = np.empty((2, C, 4 * SI, 4 * SX), dtype=np.float32)
    for core in range(N_CORES):
        n, hb, wb = core // 4, (core // 2) % 2, core % 2
        out[n, :, hb * 2 * SI: (hb + 1) * 2 * SI,
            wb * 2 * SX: (wb + 1) * 2 * SX] = (
            res.results[core]["out"].astype(np.float32))
    return out


# revision 7
# speedup vs baseline: 1.4016x; 1.0559x over previous
"""CARAFE upsampling kernel for 8 Trainium2 NeuronCores.

Problem (hardcoded): features (2,256,128,128) f32, masks (2,25,256,256) f32,
out (2,256,256,256) f32.  K=5, G=1, scale=2 (CARAFE content-aware upsample).

Strategy
--------
Sharding: 8 cores = batch(2) x H-half(2) x W-half(2).  Each core owns the
full C=256 and a 64x64 source patch (128x128 output patch) with a 2-pixel
feature halo (zero-padded on host).

Compute: ALL 25 taps of the dynamic filter are packed into a single K=120
TensorE contraction per output tile.  Tiles are (si-block B=8 source rows) x
(x-window jwin=6 source cols): the contraction axis enumerates (yrel, u) =
12 y-rows x 10 x-cols of the feature patch covering the whole tile + halo.
For tile (sb, w) and channel half ch:

    out[c, (si a jj b)] = featT[(yrel u), c]^T  @  band[(yrel u), (si jj a b)]

featT is the feature patch in [(yrel u), c] layout (host-replicated: a row
appears in ~1.4 si-blocks and a column in ~1.6 windows -> 5.4 MB vs 2.35 MB
raw; shipping the replicated layout is what lets one matmul cover all 25
taps).  band holds mask values at (yrel=si+dy, u=jj+dx), zeros elsewhere
(25 live taps of 120 rows per column).  Since the cost of a matmul scales
only with output columns (K is free), this cuts PE time 5x vs per-dy
accumulation: 32768 psum cols total ~ 14 us.

11 x-windows: 10 full (jwin=6, N=192 psum cols) + 1 edge (jwin=4, N=128,
x-window overlapping the previous one; band zeros mask the overlap).  Two
windows share a PSUM bank (384 cols); the second matmul uses start=False
and lands on cleared has_written bits, overwriting its column range.

Output: fp16 DRAM tensor (halves store traffic; adds ~1e-4 rel err).
PSUM->SBUF unscramble copies (psum (si jj a b) -> stage (si a j)) run on
DVE for ch0 and ACT for ch1; stages are 4 KB/partition stores with
contiguous 4 KB runs per channel -> full DMA bandwidth.

TimelineSim cost model: DMA ~49 us (feat 5.4 MB + bands 3.9 MB + out
8.4 MB fp16 at 360 GB/s) is the bottleneck; PE ~14-27 us hides under it.
"""

import numpy as np

import concourse.bacc as bacc
import concourse.bass as bass
import concourse.mybir as mybir
import concourse.tile as tile
from concourse.bass_utils import run_bass_kernel_spmd

FP16 = mybir.dt.float16
F32 = mybir.dt.float32

N_CORES = 8
C = 256
SI = 64           # source rows per core
SX = 64           # source cols per core
B = 8             # si-block size
NSB = SI // B     # 8 si-blocks
YW = B + 4        # 12 y-rows per block
JW = 6            # source cols per full window
W = JW + 4        # 10 x-cols per window
NWIN = 11         # 10 full + 1 edge window
KP = YW * W       # 120 contraction partitions
NCOL = 2048       # psum/stage cols per (sb, ch): 10*192 + 128
X0 = [0, 6, 12, 18, 24, 30, 36, 42, 48, 54, 58]  # window x origins (padded)

_CACHED_NC = None
TRACE = False
_LAST_RESULTS = None


def _build_nc():
    nc = bacc.Bacc(None, target_bir_lowering=False, debug=False)

    featrep_d = nc.dram_tensor("featrep", [NSB, KP, NWIN, C], FP16,
                               kind="ExternalInput")
    bands_d = nc.dram_tensor("bands", [NSB, KP, NCOL], FP16,
                             kind="ExternalInput")
    out_d = nc.dram_tensor("out", [C, 2 * SI, 2 * SX], FP16,
                           kind="ExternalOutput")

    with tile.TileContext(nc) as tc:
        with (
            tc.tile_pool(name="feat", bufs=4) as fpool,
            tc.tile_pool(name="bands", bufs=4) as bpool,
            tc.tile_pool(name="psum", bufs=8, space=bass.MemorySpace.PSUM) as ppool,
            tc.tile_pool(name="stage", bufs=4) as spool,
        ):
            for sb in range(NSB):
                ftile = fpool.tile([KP, NWIN * C], FP16, tag="ft", name=f"ft{sb}")
                btile = bpool.tile([KP, NCOL], FP16, tag="bt", name=f"bt{sb}")
                nc.sync.dma_start(
                    ftile[:], featrep_d[sb].rearrange("p w c -> p (w c)"))
                nc.sync.dma_start(btile[:], bands_d[sb])

                for ch in range(2):
                    stg = spool.tile([128, NCOL], FP16)
                    stgv = stg[:].rearrange("p (s a u) -> p s a u", s=B, a=2)
                    for wp in range(6):
                        ps = ppool.tile([128, 384], F32)
                        wis = [2 * wp, 2 * wp + 1] if wp < 5 else [10]
                        for k, wi in enumerate(wis):
                            jw = JW if wi < 10 else 4
                            ncols = 4 * B * jw
                            nc.tensor.matmul(
                                ps[:, 192 * k: 192 * k + ncols],
                                ftile[:, wi * C + ch * 128: wi * C + ch * 128 + 128],
                                btile[:, wi * 192: wi * 192 + ncols],
                                start=(k == 0),
                                stop=(k == len(wis) - 1),
                                skip_group_check=True,
                            )
                        # psum cols are (w, s, a, jb); stage cols (s, a, j).
                        # One copy per subpixel-row a keeps both APs 3D.
                        for a in range(2):
                            if wp < 5:
                                src = ps[:].rearrange(
                                    "p (w s a q) -> p s a w q",
                                    w=2, s=B, a=2, q=12)[:, :, a]
                                dst = stgv[:, :, a, 24 * wp: 24 * wp + 24].rearrange(
                                    "p s (w q) -> p s w q", w=2, q=12)
                            else:
                                src = ps[:, :128].rearrange(
                                    "p (s a q) -> p s a q",
                                    s=B, a=2, q=8)[:, :, a]
                                dst = stgv[:, :, a, 120:128]
                            if ch == 0:
                                nc.vector.tensor_copy(dst, src)
                            else:
                                nc.scalar.copy(dst, src)
                    odst = out_d[ch * 128: (ch + 1) * 128,
                                 sb * 2 * B: (sb + 1) * 2 * B, :]
                    # SWDGE store: keeps the ACT sequencer free for copies
                    # (a HWDGE store's sem wait would hold ACT SEQ) and
                    # stays off the shared HWDGE device.
                    nc.gpsimd.dma_start(odst.rearrange("c a j -> c (a j)"), stg[:])

    nc.compile()
    return nc


def _get_nc():
    global _CACHED_NC
    if _CACHED_NC is None:
        _CACHED_NC = _build_nc()
    return _CACHED_NC


def _prep_core_inputs(features: np.ndarray, masks: np.ndarray):
    fp = np.pad(features, ((0, 0), (0, 0), (2, 2), (2, 2)))

    x0 = np.array(X0)
    u = np.arange(W)
    x_uw = u[:, None] + x0[None, :]                       # (u, w) padded x
    sbi = np.arange(NSB)
    yrel = np.arange(YW)
    y_sy = 8 * sbi[:, None] + yrel[None, :]               # (sb, yrel) padded y

    wcols = np.arange(10)[:, None, None] * 192
    acols = np.arange(2)[None, :, None]
    bcols = np.arange(2)[None, None, :]

    in_maps = []
    for core in range(N_CORES):
        n, hb, wb = core // 4, (core // 2) % 2, core % 2

        fsl = fp[n, :, hb * SI: hb * SI + SI + 4, wb * SX: wb * SX + SX + 4]
        fyxc = np.ascontiguousarray(fsl.transpose(1, 2, 0))  # (68y, 68x, 256c)
        featrep = fyxc[y_sy[:, :, None, None], x_uw[None, None, :, :], :]
        featrep = featrep.reshape(NSB, KP, NWIN, C).astype(np.float16)

        msl = masks[n, :, hb * 2 * SI: (hb + 1) * 2 * SI,
                    wb * 2 * SX: (wb + 1) * 2 * SX]
        bands = np.zeros((NSB, YW, W, NCOL), np.float32)
        for dy in range(5):
            for dx in range(5):
                M = msl[5 * dy + dx].reshape(SI, 2, SX, 2)  # (S, a, xj, b)
                for si in range(B):
                    # full windows: xj = 6w + jj, w in [0,10)
                    # col = w*192 + si*24 + a*12 + jj*2 + b
                    for jj in range(JW):
                        vals = M[si::B][:, :, jj:60:JW, :]  # (sb, a, w, b)
                        cols = wcols + si * 24 + acols * 12 + jj * 2 + bcols
                        bands[:, si + dy, jj + dx, cols.ravel()] = (
                            vals.transpose(0, 2, 1, 3).reshape(NSB, -1))
                    # edge window w=10: xj = 60 + jjloc, u = 2 + jjloc + dx
                    # col = 1920 + si*16 + a*8 + jjloc*2 + b
                    for jjl in range(4):
                        vals = M[si::B][:, :, 60 + jjl, :]  # (sb, a, b)
                        cols = 1920 + si * 16 + acols[0] * 8 + jjl * 2 + bcols[0]
                        bands[:, si + dy, 2 + jjl + dx, cols.ravel()] = (
                            vals.reshape(NSB, -1))
        bands = bands.reshape(NSB, KP, NCOL).astype(np.float16)

        in_maps.append({"featrep": featrep, "bands": bands})
    return in_maps


def kernel(features: np.ndarray, masks: np.ndarray) -> np.ndarray:
    global _LAST_RESULTS
    features = np.asarray(features, dtype=np.float32)
    masks = np.asarray(masks, dtype=np.float32)

    nc = _get_nc()
    in_maps = _prep_core_inputs(features, masks)
    res = run_bass_kernel_spmd(nc, in_maps, list(range(N_CORES)), trace=TRACE)
    _LAST_RESULTS = res

    out = np.empty((2, C, 4 * SI, 4 * SX), dtype=np.float32)
    for core in range(N_CORES):
        n, hb, wb = core // 4, (core // 2) % 2, core % 2
        out[n, :, hb * 2 * SI: (hb + 1) * 2 * SI,
            wb * 2 * SX: (wb + 1) * 2 * SX] = (
            res.results[core]["out"].astype(np.float32))
    return out
